# revision 17
# baseline (speedup 1.0000x reference)
"""MoE FFN kernel, routed-sharding variant for 8 trn2 NeuronCores.

Sharding strategy (host, inside kernel()): compute the top-1 gate in exact
fp32, gather each expert's tokens, RMS-normalize, fold rms_w, cast fp16 and
transpose — each core receives its expert's compact activations x~T [D, CAP]
plus that expert's W1/W2 (fp16) and b1 (pre-transposed columns). The device
runs the expert FFN: hT = silu(W1^T @ x~T + b1); yT = W2^T-accumulated
[D, CAP]; fp16 out. Host: upcast, transpose, add b2, scale by gate score,
scatter to [B,T,D].

All weights are SBUF-resident via a few mega-DMAs (multi-dim access
patterns) so the DMA engines stream at full depth from t=0; W1 arrives in
512-col groups so MM1 starts as soon as the first group lands.
"""
import numpy as np

import concourse.bass as bass
import concourse.mybir as mybir
import concourse.tile as tile
from concourse.bacc import Bacc
from concourse.bass_utils import run_bass_kernel_spmd

B, T, D, F, E = 2, 1024, 1024, 4096, 8
N = B * T
P = 128
KD = D // P          # 8
KF = F // P          # 32
CAP = 278            # per-expert token capacity (true counts 234..277 for this input)
EPS = 1e-6

f32 = mybir.dt.float32
f16 = mybir.dt.float16
AF = mybir.ActivationFunctionType

_CACHE = {}


def build_nc():
    nc = Bacc()
    xt16 = nc.dram_tensor("xt16", [D, CAP], f16, kind="ExternalInput")
    w1 = nc.dram_tensor("w1", [D, F], f16, kind="ExternalInput")
    b1c_in = nc.dram_tensor("b1c", [P, KF], f32, kind="ExternalInput")
    w2 = nc.dram_tensor("w2", [F, D], f16, kind="ExternalInput")
    y_out = nc.dram_tensor("y", [D, CAP], f16, kind="ExternalOutput")

    with tile.TileContext(nc) as tc:
        with tc.tile_pool(name="const", bufs=1) as cst:
            # issue order follows the first matmul's needs: W1 group 0 (the
            # long pole), then the compact activations, then b1 columns
            # (needed only by the first silu), then the remaining weights.
            w1s = cst.tile([P, KD * F], f16)
            src = bass.AP(tensor=w1[:].tensor, offset=0,
                          ap=[[F, P], [P * F, KD], [1, 512]])
            nc.sync.dma_start(out=w1s[:, 0:4096], in_=src)
            xT = cst.tile([P, KD * CAP], f16)
            xt_src = bass.AP(tensor=xt16[:].tensor, offset=0,
                             ap=[[CAP, P], [P * CAP, KD], [1, CAP]])
            nc.sync.dma_start(out=xT[:], in_=xt_src)
            b1c = cst.tile([P, KF], f32)
            nc.sync.dma_start(out=b1c[:], in_=b1c_in[:])
            # remaining W1 512-col groups, col = g*4096 + kd*512 + f_local
            for g in range(1, 8):
                src = bass.AP(tensor=w1[:].tensor, offset=g * 512,
                              ap=[[F, P], [P * F, KD], [1, 512]])
                nc.sync.dma_start(out=w1s[:, g * 4096:(g + 1) * 4096], in_=src)

            def w1_block(k, kf):
                g, j = kf // 4, kf % 4
                base = g * 4096 + k * 512 + j * P
                return w1s[:, base:base + P]
            # W2 fp16 resident, col = kf*1024 + d (kf-major, 2KB lines)
            w2s = cst.tile([P, KF * D], f16)
            for g in range(4):
                src = bass.AP(tensor=w2[:].tensor, offset=g * 8 * P * D,
                              ap=[[D, P], [P * D, 8], [1, D]])
                nc.sync.dma_start(out=w2s[:, g * 8192:(g + 1) * 8192], in_=src)

            hT = cst.tile([P, KF * CAP], f16)

            # MM1: hT = silu(W1^T @ x~T + b1)
            with tc.tile_pool(name="hps", bufs=3, space="PSUM") as hps:
                for kf in range(KF):
                    hp = hps.tile([P, CAP], f32, tag="hp", name=f"hp{kf}")
                    for k in range(KD):
                        nc.tensor.matmul(
                            out=hp[:],
                            lhsT=w1_block(k, kf),
                            rhs=xT[:, k * CAP:(k + 1) * CAP],
                            start=(k == 0), stop=(k == KD - 1),
                        )
                    nc.scalar.activation(
                        out=hT[:, kf * CAP:(kf + 1) * CAP], in_=hp[:],
                        func=AF.Silu, bias=b1c[:, kf:kf + 1], scale=1.0,
                    )

            # MM2: yT[d, slot] accumulated over F, in two halves of 4 d-blocks
            # (kf-outer, m-inner bank rotation within each half) so the first
            # half's casts + output DMAs overlap the second half's matmuls.
            with (
                tc.tile_pool(name="yout", bufs=2) as yp,
                tc.tile_pool(name="yps", bufs=1, space="PSUM") as yps,
            ):
                for half in range(2):
                    ms = range(half * 4, half * 4 + 4)
                    ypss = {m: yps.tile([P, CAP], f32, tag=f"y{m}", name=f"ypss{m}") for m in ms}
                    for kf in range(KF):
                        for m in ms:
                            nc.tensor.matmul(
                                out=ypss[m][:],
                                lhsT=w2s[:, kf * 1024 + m * P:kf * 1024 + (m + 1) * P],
                                rhs=hT[:, kf * CAP:(kf + 1) * CAP],
                                start=(kf == 0), stop=(kf == KF - 1),
                                skip_group_check=True,
                            )
                    for m in ms:
                        ysb = yp.tile([P, CAP], f16, tag="ysb", name=f"ysb{m}")
                        nc.vector.tensor_copy(out=ysb[:], in_=ypss[m][:])
                        nc.sync.dma_start(out=y_out[m * P:(m + 1) * P, :], in_=ysb[:])

    nc.finalize()
    return nc


def _route(x, rms_w, gate_w):
    """Host gate: exact fp32 RMSNorm + top-1 routing (matches reference)."""
    x2d = np.asarray(x, np.float32).reshape(N, D)
    rms = np.asarray(rms_w, np.float32)
    ms = np.mean(x2d * x2d, axis=1, keepdims=True)
    xn = x2d * (1.0 / np.sqrt(ms + EPS)) * rms[None, :]
    logits = xn @ np.asarray(gate_w, np.float32).T
    idx = np.argmax(logits, axis=1)
    m = logits.max(axis=1, keepdims=True)
    score = (1.0 / np.exp(logits - m).sum(axis=1)).astype(np.float32)
    return xn, idx, score


def make_in_maps(x, rms_w, gate_w, W1, b1, W2, b2):
    xn, idx, score = _route(x, rms_w, gate_w)
    in_maps = []
    routing = []
    for c in range(E):
        toks = np.where(idx == c)[0]
        spill = toks[CAP:]          # capacity overflow -> host FFN (never for
        toks = toks[:CAP]           # the expected token distribution)
        xt = np.zeros((D, CAP), np.float16)
        xt[:, :len(toks)] = xn[toks].astype(np.float16).T
        in_maps.append({
            "xt16": np.ascontiguousarray(xt),
            "w1": np.ascontiguousarray(np.asarray(W1[c], np.float16)),
            "b1c": np.ascontiguousarray(np.asarray(b1[c], np.float32).reshape(KF, P).T),
            "w2": np.ascontiguousarray(np.asarray(W2[c], np.float16)),
        })
        routing.append((toks, score[toks], spill, score[spill], xn[spill]))
    return in_maps, routing


def combine(results, routing, W1, b1, W2, b2):
    out = np.zeros((N, D), np.float32)
    for c in range(E):
        yT = results[c]["y"].astype(np.float32)   # [D, CAP]
        toks, score, spill, sscore, sxn = routing[c]
        b2c = np.asarray(b2[c], np.float32)
        out[toks] = (yT.T[:len(toks)] + b2c[None, :]) * score[:, None]
        if len(spill):
            h = sxn @ np.asarray(W1[c], np.float32) + np.asarray(b1[c], np.float32)
            h *= 1.0 / (1.0 + np.exp(-h))
            ys = h @ np.asarray(W2[c], np.float32) + b2c
            out[spill] = ys * sscore[:, None]
    return out.reshape(B, T, D)


def kernel(x, rms_w, gate_w, W1, b1, W2, b2, **_):
    if "nc" not in _CACHE:
        _CACHE["nc"] = build_nc()
    nc = _CACHE["nc"]
    in_maps, routing = make_in_maps(x, rms_w, gate_w, W1, b1, W2, b2)
    res = run_bass_kernel_spmd(nc, in_maps, list(range(E)))
    return combine(res.results, routing, W1, b1, W2, b2)
